# revision 1
# baseline (speedup 1.0000x reference)
"""Balanced EMD loss kernel for Trainium2 (8 NeuronCores, data parallel).

Math (per sample, classes w = 1..10):
    score = sum(pt * w);  var = sum(pt * (w - score)^2) = Z2 - Z1^2  (S0 ~= 1)
    cdf_diff = cumsum(pe) - cumsum(pt) = cumsum(pe - pt)
    emd = sqrt(mean(cdf_diff^2));  loss = sum(emd / var) / B

Layout: samples distributed over 128 partitions; each partition holds a
contiguous run of samples, 10 classes contiguous in the free dim.

Engine split per tile:
  VectorE: q = pe - pt; masked scan (per-sample cumsum via a periodic 0/1
           multiplicative reset pattern); per-sample reduce of cdf^2;
           small finishing ops (var, 1/var, loss accumulate).
  ScalarE: square of the cdf (in place) and PSUM->SBUF moves for the
           TensorE moment pipeline; final sqrt.
  TensorE: weighted moments Z1 = sum(pt*w), Z2 = sum(pt*w^2): transpose
           [128,120] chunks to class-on-partition, block-diagonal [120,24]
           matmul, transpose the [24,128] results back to a dense
           [128, samples*2] layout.
"""

import numpy as np

P = 128          # SBUF partitions
C = 10           # classes
K = 396          # samples per partition per tile (multiple of 12)
NT = 10          # tiles
KT = K * NT      # samples per partition per core
SHARD = P * KT   # padded rows per core
NCORES = 8
PAD_VAL = 0.1    # pt == pe == 0.1 -> emd == 0 -> zero loss contribution

SLOT = 12        # samples per transpose chunk (120 = SLOT*C free positions)
GCH = 3          # chunks per matmul group (PSUM bank holds 3*128 = 384 cols)

_CACHE = {}


def _build_nc(k=K, nt=NT):
    import concourse.bass as bass
    import concourse.tile as tile
    from concourse import bacc, mybir

    dt = mybir.dt.float32
    dth = mybir.dt.float16
    Alu = mybir.AluOpType
    F = k * C
    n_chunk = k // SLOT              # transpose chunks per tile
    n_group = n_chunk // GCH         # matmul groups per tile
    CW = SLOT * C                    # 120 free positions per chunk
    GW = GCH * P                     # matmul group column count (384)
    MW = GCH * 2 * SLOT              # momd free elems per group (72)

    nc = bacc.Bacc("TRN2")
    pt_d = nc.dram_tensor("pt", [P, k * nt, C], dth, kind="ExternalInput").ap()
    pe_d = nc.dram_tensor("pe", [P, k * nt, C], dth, kind="ExternalInput").ap()
    msk_d = nc.dram_tensor("mask01", [P, F], dth, kind="ExternalInput").ap()
    w_d = nc.dram_tensor("wst", [CW, 2 * SLOT], dth, kind="ExternalInput").ap()
    id_d = nc.dram_tensor("ident", [P, P], dth, kind="ExternalInput").ap()
    out_d = nc.dram_tensor("out", [P, nt + 1], dt, kind="ExternalOutput").ap()

    with tile.TileContext(nc) as tc:
        with (
            tc.tile_pool(name="consts", bufs=1) as cpool,
            tc.tile_pool(name="ins", bufs=4) as ipool,
            tc.tile_pool(name="mm", bufs=4) as mpool,
            tc.tile_pool(name="small", bufs=3) as spool,
            tc.tile_pool(name="ps1", bufs=4, space="PSUM") as ppool1,
            tc.tile_pool(name="ps2", bufs=2, space="PSUM") as ppool2,
            tc.tile_pool(name="ps3", bufs=2, space="PSUM") as ppool3,
            tc.tile_pool(name="outp", bufs=1) as opool,
        ):
            # tile schedule: two warmup half-tiles shorten the initial DVE
            # stall; their input DMAs are issued before the const DMAs
            k1 = (k // 2 // SLOT) * SLOT
            if k1 >= SLOT and k - k1 >= SLOT:
                tiles = [(0, k1), (k1, k - k1)]
            else:
                tiles = [(0, k)]
            off0 = tiles[-1][0] + tiles[-1][1]
            tiles += [(o, k) for o in range(off0, k * nt, k)]

            def load(off, ki):
                ptt = ipool.tile([P, F], dth, tag="ptt")
                nc.sync.dma_start(
                    ptt[:, : ki * C].rearrange("p (k c) -> p k c", c=C),
                    pt_d[:, off : off + ki, :],
                )
                pet = ipool.tile([P, F], dth, tag="pet")
                nc.sync.dma_start(
                    pet[:, : ki * C].rearrange("p (k c) -> p k c", c=C),
                    pe_d[:, off : off + ki, :],
                )
                return ptt, pet

            preload = load(*tiles[0])

            cmask = cpool.tile([P, F], dth, tag="cmask")
            nc.sync.dma_start(cmask[:], msk_d[:])
            wst = cpool.tile([CW, 2 * SLOT], dth, tag="wst")
            nc.sync.dma_start(wst[:], w_d[:])
            ident = cpool.tile([P, P], dth, tag="ident")
            nc.sync.dma_start(ident[:], id_d[:])

            acc = opool.tile([P, len(tiles)], dt, tag="acc")

            for i, (off, ki) in enumerate(tiles):
                fi = ki * C
                ptt, pet = preload if i == 0 else load(off, ki)

                # ---- VectorE cdf pipeline ----
                # q = pe - pt  (in place on the pe tile)
                nc.vector.tensor_sub(pet[:, :fi], pet[:, :fi], ptt[:, :fi])
                # per-sample cumsum: state = mask*state + q, in place
                nc.vector.tensor_tensor_scan(
                    pet[:, :fi], cmask[:, :fi], pet[:, :fi], 0.0,
                    op0=Alu.mult, op1=Alu.add,
                )
                # square on the scalar engine, in place
                nc.scalar.square(pet[:, :fi], pet[:, :fi])
                # ssq = sum over classes of cdf_diff^2
                ssqm = spool.tile([P, k], dt, tag="ssqm")
                nc.vector.tensor_reduce(
                    ssqm[:, :ki],
                    pet[:, :fi].rearrange("p (k c) -> p k c", c=C),
                    axis=mybir.AxisListType.X,
                    op=Alu.add,
                )

                # ---- TensorE moment pipeline over pt ----
                # transpose [128,120] chunks to class-on-partition, then
                # matmul with the chunk as STATIONARY and the block-diag
                # weight matrix as moving: out = sb_chunk^T @ wst =
                # [128 samples, 24] -- moments, already dense.
                nchk = ki // SLOT
                ngrp = (nchk + GCH - 1) // GCH
                n_half = (nchk + 1) // 2  # chunks in first PSUM bank
                nha = (n_chunk + 1) // 2  # max bank-a chunk capacity
                momd = mpool.tile([P, 2 * k], dt, tag="momd")
                mdp_a = ppool2.tile([P, nha * 2 * SLOT], dt, tag="mdp_a")
                mdp_b = ppool3.tile(
                    [P, (n_chunk - nha) * 2 * SLOT], dt, tag="mdp_b"
                )
                for g in range(ngrp):
                    gch = min(GCH, nchk - g * GCH)
                    pst = ppool1.tile([CW, GW], dth, tag="pst")
                    for j in range(gch):
                        ch = g * GCH + j
                        nc.tensor.transpose(
                            pst[:, bass.ts(j, P)],
                            ptt[:, bass.ts(ch, CW)],
                            ident[:],
                        )
                    sb = mpool.tile([CW, GW], dth, tag="sb")
                    nc.scalar.copy(sb[:, : gch * P], pst[:, : gch * P])
                    for j in range(gch):
                        ch = g * GCH + j
                        dst = (
                            mdp_a[:, bass.ts(ch, 2 * SLOT)]
                            if ch < n_half
                            else mdp_b[:, bass.ts(ch - n_half, 2 * SLOT)]
                        )
                        nc.tensor.matmul(
                            dst, sb[:, bass.ts(j, P)], wst[:],
                            start=True, stop=True,
                        )
                nc.scalar.copy(
                    momd[:, : n_half * 2 * SLOT], mdp_a[:, : n_half * 2 * SLOT]
                )
                if nchk > n_half:
                    nc.scalar.copy(
                        momd[:, n_half * 2 * SLOT : nchk * 2 * SLOT],
                        mdp_b[:, : (nchk - n_half) * 2 * SLOT],
                    )

                # ---- finishing ----
                # momd free layout: (chunk, slot, mtype) -> sample index
                # 12*chunk + slot; mtype 0 -> Z1/16, 1 -> Z2/256
                z1 = momd[:, : 2 * ki].rearrange("p (k m) -> p k m", m=2)[:, :, 0]
                z2 = momd[:, : 2 * ki].rearrange("p (k m) -> p k m", m=2)[:, :, 1]
                tv = spool.tile([P, k], dt, tag="tv")
                # var = 256*z2 - 256*z1^2   (z1 = Z1/16, z2 = Z2/256)
                nc.vector.scalar_tensor_tensor(
                    tv[:, :ki], z1, -256.0, z1, op0=Alu.mult, op1=Alu.mult
                )
                nc.vector.scalar_tensor_tensor(
                    tv[:, :ki], z2, 256.0, tv[:, :ki], op0=Alu.mult, op1=Alu.add
                )
                nc.vector.reciprocal_approx_fast(tv[:, :ki], tv[:, :ki])
                # emd = sqrt(ssq/10), in place on ssqm
                nc.scalar.activation(
                    ssqm[:, :ki], ssqm[:, :ki],
                    mybir.ActivationFunctionType.Sqrt, scale=0.1,
                )
                # acc[:, i] = sum_k emd * wgt
                nc.vector.tensor_mul(tv[:, :ki], ssqm[:, :ki], tv[:, :ki])
                nc.vector.tensor_reduce(
                    acc[:, i : i + 1], tv[:, :ki],
                    axis=mybir.AxisListType.X, op=Alu.add,
                )

            nc.sync.dma_start(out_d[:, : len(tiles)], acc[:])

    nc.compile()
    return nc


def _consts(k=K):
    F = k * C
    m01 = np.tile(np.array([0.0] + [1.0] * (C - 1), np.float16), k)
    mask_full = np.ascontiguousarray(np.broadcast_to(m01, (P, F)))

    # block-diagonal stationary, fp16-exact dyadic weights: for slot s,
    # class c: wst[10s+c, 2s] = (c+1)/16 -> Z1/16;
    #          wst[10s+c, 2s+1] = (c+1)^2/256 -> Z2/256
    wst = np.zeros((SLOT * C, 2 * SLOT), np.float16)
    wv1 = (np.arange(1, C + 1, dtype=np.float64) / 16.0).astype(np.float16)
    wv2 = (np.arange(1, C + 1, dtype=np.float64) ** 2 / 256.0).astype(np.float16)
    for s in range(SLOT):
        wst[10 * s : 10 * s + 10, 2 * s] = wv1
        wst[10 * s : 10 * s + 10, 2 * s + 1] = wv2

    ident = np.eye(P, dtype=np.float16)
    return mask_full, wst, ident


def _shards(x, per, shard_rows):
    out = []
    for i in range(NCORES):
        s = x[i * per : (i + 1) * per]
        pad = shard_rows - s.shape[0]
        if pad:
            s = np.concatenate([s, np.full((pad, C), PAD_VAL, x.dtype)], axis=0)
        out.append(np.ascontiguousarray(s.reshape(P, shard_rows // P, C)))
    return out


def kernel(p_target: np.ndarray, p_estimate: np.ndarray) -> np.ndarray:
    from concourse.bass_utils import run_bass_kernel_spmd

    if "nc" not in _CACHE:
        _CACHE["nc"] = _build_nc()
    nc = _CACHE["nc"]

    B = p_target.shape[0]
    per = B // NCORES
    mask_full, wst, ident = _consts()
    pt_sh = _shards(np.asarray(p_target).astype(np.float16), per, SHARD)
    pe_sh = _shards(np.asarray(p_estimate).astype(np.float16), per, SHARD)

    in_maps = [
        {
            "pt": pt_sh[i],
            "pe": pe_sh[i],
            "mask01": mask_full,
            "wst": wst,
            "ident": ident,
        }
        for i in range(NCORES)
    ]
    res = run_bass_kernel_spmd(nc, in_maps, core_ids=list(range(NCORES)))
    total = sum(
        res.results[i]["out"].astype(np.float64).sum() for i in range(NCORES)
    )
    return np.float32(total / B)



# revision 21
# speedup vs baseline: 2.3122x; 2.3122x over previous
"""Balanced EMD loss kernel for Trainium2 (8 NeuronCores, data parallel).

Math (per sample, classes w = 1..10):
    d_k   = cumsum(pe - pt)[k], k = 0..8   (d_9 == 0 exactly in the reference)
    emd   = sqrt(sum_k d_k^2 / 10)
    var   = E[w^2] - score^2 = 16*(z2 - z1^2),  z1 = score/4, z2 = E[w^2]/16
    loss  = sum(emd / var) / B

Device pipeline (per core, fp8-e4m3 inputs):
  Host packs pt/pe into a block-interleaved layout [2, 120, NB] where
  partition row r = slot*10 + class, block m holds samples 12m+slot.
  TensorE (DoubleRow fp8 matmuls, data as stationary): one matmul per
  128-block chunk emits the 9 cdf-diffs d_k per sample straight into
  PSUM; two accumulating matmuls emit z1 and z2 (z2 split into two
  fp8-exact weight columns A/16 + B/16, summed in PSUM).
  ScalarE/GpSimd split the d^2 squaring pass (PSUM fp32 -> SBUF fp16)
  by column ranges; VectorE reduces the 9 squares per sample with a
  4-pass fp16 add tree; z1^2 (ScalarE), v = z2 - z1^2 (GpSimd), and the
  final sum of (emd/16)/v via a dividing scalar_tensor_tensor accumulate
  (VectorE) are batched over 5-span super-spans and software-pipelined
  one span behind the producers.
"""

import numpy as np

C = 10
SLOT = 12
R = 120                  # partition rows = SLOT * C
CB = 128                 # blocks per chunk (matmul stationary columns)
SPAN = 8                 # chunks per span: 2 PSUM d-banks
MSPAN = 4                # spans per super-span: 2-bank m-tile, tail batch
NCH = 330                # chunks per core
NB = NCH * CB            # 42240 blocks per core
SAMP = NB * SLOT         # 506880 padded samples per core
NCORES = 8
PAD_VAL = 0.1
DMA_SPANS = 1            # spans per steady-state input DMA transfer
SA = 432                 # Act square share (Act only: single-PSUM-input rule)
DCOPY = 0                # cols per bank squared on DVE via copy+square
PP = 432                 # GpSimd square share end: cols [SA, min(SA+PP,432))
USE_DIV = False          # ALU divide fails DVE ISA check on hw; use reciprocal
TAILSPLIT = True         # last group -> single-span groups (shorter drain)
Z1_ON_DVE = False        # z1^2 on VectorE instead of ScalarE
FINAL_ON_POOL = False    # final accumulate on GpSimd instead of VectorE
IBUFS = 6
SQBUFS = 3
SBUFS = 4
DCOLS = 108              # d columns per chunk (12 samples x 9 cdf positions)
MCOLS = 24               # moment columns per chunk (12 z1 + 12 z2)

AW = [1, 4, 9, 16, 24, 36, 48, 64, 80, 96]   # e4m3-exact part of w^2
BW = [0, 0, 0, 0, 1, 0, 1, 0, 1, 4]          # w^2 - AW (also e4m3-exact)

_CACHE = {}


def _spans(nch):
    out, o = [], 0
    while o < nch:
        g = min(SPAN, nch - o)
        out.append((o, g))
        o += g
    return out


def _build_nc(nch=NCH):
    import concourse.tile as tile
    from concourse import bacc, mybir

    f32, f16, f8 = mybir.dt.float32, mybir.dt.float16, mybir.dt.float8e4
    Alu = mybir.AluOpType
    AF = mybir.ActivationFunctionType
    DR = mybir.MatmulPerfMode.DoubleRow

    nb = nch * CB
    spans = _spans(nch)
    # group full spans MSPAN at a time; a short (ragged) span gets its own
    # group so each group's squared-cdf buffer stays contiguous
    sspans, cur = [], []
    for sp in spans:
        if sp[1] == SPAN:
            cur.append(sp)
            if len(cur) == MSPAN:
                sspans.append(cur)
                cur = []
        else:
            if cur:
                sspans.append(cur)
                cur = []
            sspans.append([sp])
    if cur:
        sspans.append(cur)
    if TAILSPLIT and len(sspans) > 1 and len(sspans[-1]) > 1:
        last = sspans.pop()
        sspans.extend([sp] for sp in last)

    # input DMA granularity: first span alone (fast pipeline start), then
    # DMA_SPANS spans per transfer
    dmas = []            # (block_start, block_count)
    span2dma = []        # span idx -> (dma idx, block offset inside dma)
    i = 0
    while i < len(spans):
        n = 1 if i == 0 else DMA_SPANS
        grp = spans[i : i + n]
        b0 = grp[0][0] * CB
        bn = sum(g for _, g in grp) * CB
        off = 0
        for _, g in grp:
            span2dma.append((len(dmas), off))
            off += g * CB
        dmas.append((b0, bn))
        i += len(grp)

    nc = bacc.Bacc("TRN2")
    x_d = nc.dram_tensor("x", [2, R, nb], f8, kind="ExternalInput").ap()
    wd_d = nc.dram_tensor("wd", [R, 2 * DCOLS], f8, kind="ExternalInput").ap()
    wm1_d = nc.dram_tensor("wm1", [R, 2 * MCOLS], f8, kind="ExternalInput").ap()
    wm2_d = nc.dram_tensor("wm2", [R, 2 * MCOLS], f8, kind="ExternalInput").ap()
    out_d = nc.dram_tensor("out", [128, len(sspans)], f32, kind="ExternalOutput").ap()

    NBK = (SPAN + 3) // 4          # psum banks per d-tile
    GMAX = MSPAN * SPAN * 12       # max sample groups per super-span

    with tile.TileContext(nc) as tc:
        with (
            tc.tile_pool(name="consts", bufs=1) as cpool,
            tc.tile_pool(name="ins", bufs=IBUFS) as ipool,
            tc.tile_pool(name="dps", bufs=2, space="PSUM") as dpool,
            tc.tile_pool(name="mps", bufs=2, space="PSUM") as mpool,
            tc.tile_pool(name="sq", bufs=SQBUFS) as sqpool,
            tc.tile_pool(name="small", bufs=SBUFS) as spool,
            tc.tile_pool(name="outp", bufs=1) as opool,
        ):
            in_tiles = {}

            def load_dma(di):
                b0, bn = dmas[di]
                t = ipool.tile([R, 2 * bn], f8, tag="xin")
                nc.sync.dma_start(
                    t.rearrange("p (i f) -> p i f", i=2),
                    x_d[:, :, b0 : b0 + bn].rearrange("i p f -> p i f"),
                )
                in_tiles[di] = t

            load_dma(0)

            wd_t = cpool.tile([R, 2 * DCOLS], f8, tag="wd")
            nc.sync.dma_start(wd_t[:], wd_d[:])
            wm1_t = cpool.tile([R, 2 * MCOLS], f8, tag="wm1")
            nc.sync.dma_start(wm1_t[:], wm1_d[:])
            wm2_t = cpool.tile([R, 2 * MCOLS], f8, tag="wm2")
            nc.sync.dma_start(wm2_t[:], wm2_d[:])

            wd_ap = wd_t.rearrange("p (i f) -> p i f", i=2)
            wm1_ap = wm1_t.rearrange("p (i f) -> p i f", i=2)
            wm2_ap = wm2_t.rearrange("p (i f) -> p i f", i=2)

            acc = opool.tile([128, len(sspans)], f32, tag="acc")

            si_global = 0
            pend = []          # deferred tail phases, one per later span slot

            def run_tail_phase():
                if pend:
                    pend.pop(0)()

            for ssi, grp in enumerate(sspans):
                NCHS = sum(g for _, g in grp)        # chunks in this group
                S = NCHS * DCOLS                     # squared-cdf cols
                G = S // 9                           # samples per partition
                sq16 = sqpool.tile([128, GMAX * 9], f16, tag="sq16")
                mt = mpool.tile([128, MSPAN * SPAN * MCOLS], f32, tag="mt")

                for si, (och, g) in enumerate(grp):
                    di, boff = span2dma[si_global]
                    si_global += 1
                    if di + 1 < len(dmas) and di + 1 not in in_tiles:
                        load_dma(di + 1)
                    it = in_tiles[di].rearrange("p (i f) -> p i f", i=2)

                    dt_ = dpool.tile([128, NBK * 512], f32, tag="dt")

                    for j in range(g):
                        lhsT = it[:, :, boff + j * CB : boff + (j + 1) * CB]
                        od = (j // 4) * 512 + (j % 4) * DCOLS
                        nc.tensor.matmul(
                            dt_[:, od : od + DCOLS], lhsT, wd_ap,
                            start=True, stop=True, perf_mode=DR,
                        )
                        om = (si * SPAN + j) * MCOLS
                        nc.tensor.matmul(
                            mt[:, om : om + MCOLS], lhsT, wm1_ap,
                            start=True, stop=False, perf_mode=DR,
                        )
                        nc.tensor.matmul(
                            mt[:, om : om + MCOLS], lhsT, wm2_ap,
                            start=False, stop=True, perf_mode=DR,
                        )

                    # squared cdf diffs: PSUM fp32 -> SBUF fp16, split by
                    # flattened column ranges: Act [0, SA), GpSimd [SA, end)
                    spq = si * SPAN * DCOLS
                    fb, rem = g // 4, (g % 4) * DCOLS
                    if fb:
                        ind = dt_[:, : fb * 512].rearrange("p (b x) -> p b x", x=512)
                        outd = sq16[:, spq : spq + fb * 432].rearrange(
                            "p (b x) -> p b x", x=432
                        )
                        sa = min(SA, 432 - DCOPY)
                        nc.scalar.square(outd[:, :, :sa], ind[:, :, :sa])
                        if DCOPY:
                            # DVE: copy PSUM->SBUF fp16 (one PSUM input),
                            # then square in SBUF at 2x fp16 rate
                            cp = spool.tile([128, 2 * 512], f16, tag="cp")
                            cpv = cp[:, : fb * DCOPY].rearrange(
                                "p (b x) -> p b x", x=DCOPY
                            )
                            nc.vector.tensor_scalar(
                                cpv, ind[:, :, sa : sa + DCOPY], 1.0, None,
                                op0=Alu.mult,
                            )
                            nc.vector.tensor_mul(
                                outd[:, :, sa : sa + DCOPY], cpv, cpv
                            )
                    if rem:
                        pin = dt_[:, fb * 512 : fb * 512 + rem]
                        pout = sq16[:, spq + fb * 432 : spq + fb * 432 + rem]
                        nc.scalar.square(pout, pin)

                    run_tail_phase()

                # tail phases for this group, spread over the next group's
                # span slots. GpSimd touches only SBUF (hw restriction).
                st = {}

                def t_p0(mt=mt, sq16=sq16, S=S, G=G, NCHS=NCHS, st=st):
                    mk = mt[:, : NCHS * MCOLS].rearrange(
                        "p (k m) -> p k m", m=MCOLS
                    )
                    z1sq = spool.tile([128, GMAX], f32, tag="z1sq")
                    z1k = z1sq[:, :G].rearrange("p (k m) -> p k m", m=12)
                    nc.scalar.square(z1k, mk[:, :, :12])
                    st["mk"], st["z1k"] = mk, z1k
                    g9 = sq16[:, :S].rearrange("p (g k) -> p g k", k=9)
                    st["g9"] = g9
                    t1 = spool.tile([128, GMAX * 4], f16, tag="t1")
                    t14 = t1[:, : G * 4].rearrange("p (g k) -> p g k", k=4)
                    nc.vector.tensor_add(t14, g9[:, :, 0:4], g9[:, :, 4:8])
                    st["t14"] = t14

                def t_p1(G=G, st=st):
                    v3 = spool.tile([128, GMAX], f32, tag="v3")
                    vk = v3[:, :G].rearrange("p (k m) -> p k m", m=12)
                    nc.vector.scalar_tensor_tensor(
                        vk, st["mk"][:, :, 12:24], 1.0, st["z1k"],
                        op0=Alu.mult, op1=Alu.subtract,
                    )
                    st["v3"] = v3

                def t_p2(G=G, st=st):
                    if not USE_DIV:
                        u3 = spool.tile([128, GMAX], f32, tag="u3")
                        nc.vector.reciprocal_approx_fast(
                            u3[:, :G], st["v3"][:, :G]
                        )
                        st["v3"] = u3
                    t14 = st["t14"]
                    t2 = spool.tile([128, GMAX * 2], f16, tag="t2")
                    t22 = t2[:, : G * 2].rearrange("p (g k) -> p g k", k=2)
                    nc.vector.tensor_add(t22, t14[:, :, 0:2], t14[:, :, 2:4])
                    t3 = spool.tile([128, GMAX], f16, tag="t3")
                    nc.vector.tensor_add(t3[:, :G], t22[:, :, 0], t22[:, :, 1])
                    st["t3"] = t3

                def t_p3(G=G, st=st, ssi=ssi):
                    ssq = spool.tile([128, GMAX], f16, tag="ssq")
                    nc.vector.tensor_add(
                        ssq[:, :G], st["t3"][:, :G], st["g9"][:, :, 8]
                    )
                    emd = spool.tile([128, GMAX], f16, tag="emd")
                    nc.scalar.activation(
                        emd[:, :G], ssq[:, :G], AF.Sqrt, scale=0.1
                    )
                    junk = spool.tile([128, GMAX], f32, tag="junk")
                    nc.vector.scalar_tensor_tensor(
                        junk[:, :G], emd[:, :G], 1.0 / 16.0, st["v3"][:, :G],
                        op0=Alu.mult,
                        op1=Alu.divide if USE_DIV else Alu.mult,
                    )
                    nc.vector.tensor_reduce(
                        acc[:, ssi : ssi + 1], junk[:, :G],
                        axis=mybir.AxisListType.X, op=Alu.add,
                    )

                nslots = max(len(grp), 2)
                phases = [t_p0, t_p1, t_p2, t_p3]
                if nslots >= 4:
                    phases = phases[:3] + [lambda: None] * (nslots - 4) + phases[3:]
                else:
                    if nslots == 3:
                        p1, p2 = t_p1, t_p2
                        phases = [t_p0, lambda: (p1(), p2()), t_p3]
                    else:
                        p0, p1, p2 = t_p0, t_p1, t_p2
                        phases = [lambda: (p0(), p1()), lambda: (p2(), t_p3())]
                pend.extend(phases)

            while pend:
                run_tail_phase()
            nc.sync.dma_start(out_d[:], acc[:])

    nc.compile()
    return nc


def _weights():
    import ml_dtypes

    F8 = ml_dtypes.float8_e4m3
    wd = np.zeros((R, 2, DCOLS), np.float32)
    wm1 = np.zeros((R, 2, MCOLS), np.float32)
    wm2 = np.zeros((R, 2, MCOLS), np.float32)
    for s in range(SLOT):
        for c in range(C):
            r = s * C + c
            for k in range(c, 9):
                wd[r, 0, s * 9 + k] = -1.0
                wd[r, 1, s * 9 + k] = 1.0
            wm1[r, 0, s] = (c + 1) / 4.0
            wm1[r, 0, SLOT + s] = AW[c] / 16.0
            wm2[r, 0, SLOT + s] = BW[c] / 16.0
    return (
        wd.reshape(R, 2 * DCOLS).astype(F8),
        wm1.reshape(R, 2 * MCOLS).astype(F8),
        wm2.reshape(R, 2 * MCOLS).astype(F8),
    )


def _pack(X, c, per, F8):
    sl = np.asarray(X[c * per : (c + 1) * per], np.float32).astype(F8)
    pad = SAMP - sl.shape[0]
    if pad:
        padrows = np.full((pad, C), PAD_VAL, np.float32).astype(F8)
        sl = np.concatenate([sl, padrows], axis=0)
    return np.ascontiguousarray(
        sl.reshape(NB, SLOT, C).transpose(1, 2, 0).reshape(R, NB)
    )


def kernel(p_target: np.ndarray, p_estimate: np.ndarray) -> np.ndarray:
    import ml_dtypes
    from concourse.bass_utils import run_bass_kernel_spmd

    F8 = ml_dtypes.float8_e4m3
    if "nc" not in _CACHE:
        _CACHE["nc"] = _build_nc()
    nc = _CACHE["nc"]

    B = p_target.shape[0]
    per = B // NCORES
    wd, wm1, wm2 = _weights()
    in_maps = []
    for c in range(NCORES):
        x = np.empty((2, R, NB), dtype=F8)
        x[0] = _pack(p_target, c, per, F8)
        x[1] = _pack(p_estimate, c, per, F8)
        in_maps.append({"x": x, "wd": wd, "wm1": wm1, "wm2": wm2})

    res = run_bass_kernel_spmd(nc, in_maps, core_ids=list(range(NCORES)))
    total = sum(
        res.results[i]["out"].astype(np.float64).sum() for i in range(NCORES)
    )
    return np.float32(total / B)
